# revision 1
# baseline (speedup 1.0000x reference)
"""CrossTransformerBlock3D Trainium2 kernel.

Shards the D axis (32) into 8 slabs of 4 (= one window depth) across the 8
NeuronCores; windows are independent so each core runs the full block on its
slab with no collectives.

Per-core dataflow (16384 tokens, 256 windows of 64 tokens, groups of 8
windows = 512 tokens):
  - token-major LN1 (bn_stats) -> bf16 -> PE-transpose to feature-major
  - q/k feature-major [heads*32, tok], v token-major via xnT-as-lhsT
  - scores^T = k @ q^T per (window, head) packed into [128, 512] PSUM tiles
    (partition slot = head parity), bias added via DVE broadcast-AP,
    softmax without max-subtraction (scores are tiny), exp on ACT,
    row sums via ones-matmul, normalization via E-matrix broadcast matmul
  - proj feature-major, PE-transpose back, residual in fp32, LN2,
    MLP feature-major with fused GELU on the PSUM eviction, final residual.
All matmuls in bf16 (fp32 PSUM accumulation): ~3e-4 relative error.
"""

import math
import numpy as np
import ml_dtypes

import concourse.bass as bass
import concourse.tile as tile
from concourse import bacc, mybir
from concourse.bass_utils import run_bass_kernel_spmd

F32 = mybir.dt.float32
BF16 = mybir.dt.bfloat16
AF = mybir.ActivationFunctionType
ALU = mybir.AluOpType

# Problem shape (hardcoded per contract)
B, D, H, W, C = 1, 32, 64, 64, 192
NH, HD = 6, 32
SCALE = HD ** -0.5
N_CORES = 8
DS = D // N_CORES            # 4 depth per core = one window depth
NWH, NWW = H // 4, W // 4    # 16 x 16 windows per core
N_WIN = NWH * NWW            # 256 windows/core
GROUP_WIN = 8                # windows per group (512 tokens)
N_GROUPS = N_WIN // GROUP_WIN  # 32
FFN = 4 * C                  # 768


def _rel_index():
    ws = (4, 4, 4)
    coords = np.stack(np.meshgrid(np.arange(ws[0]), np.arange(ws[1]), np.arange(ws[2]), indexing='ij'))
    cf = coords.reshape(3, -1)
    rel = (cf[:, :, None] - cf[:, None, :]).transpose(1, 2, 0).copy()
    rel[:, :, 0] += ws[0] - 1
    rel[:, :, 1] += ws[1] - 1
    rel[:, :, 2] += ws[2] - 1
    rel[:, :, 0] *= (2 * ws[1] - 1) * (2 * ws[2] - 1)
    rel[:, :, 1] *= 2 * ws[2] - 1
    return rel.sum(-1)


def bf16(a):
    return np.asarray(a, np.float32).astype(ml_dtypes.bfloat16)


def _win_src_ap(dram_ap, g, u):
    """AP for the 128 tokens of window pair (8g+2u, 8g+2u+1); inputs are
    host-permuted to window-token order [16384, 192]."""
    t0 = (g * GROUP_WIN + 2 * u) * 64
    return dram_ap[t0:t0 + 128, :]


def win_permute(slab):
    """[DS,H,W,C] -> [N_WIN*64, C] in (wh, ww, d, i, j) token order."""
    t = slab.reshape(DS, NWH, 4, NWW, 4, C).transpose(1, 3, 0, 2, 4, 5)
    return np.ascontiguousarray(t.reshape(N_WIN * 64, C))


def win_unpermute(flat):
    """[N_WIN*64, C] -> [DS,H,W,C]."""
    t = flat.reshape(NWH, NWW, DS, 4, 4, C).transpose(2, 0, 3, 1, 4, 5)
    return np.ascontiguousarray(t.reshape(DS, H, W, C))


def build_program(weights_np):
    """Build the per-core Bass program. weights_np: dict of host-prepped arrays."""
    nc = bacc.Bacc("TRN2", target_bir_lowering=False, debug=False)

    xs = nc.dram_tensor("xs", [N_WIN * 64, C], F32, kind="ExternalInput").ap()
    ys = nc.dram_tensor("ys", [N_WIN * 64, C], F32, kind="ExternalInput").ap()
    wq = nc.dram_tensor("wq", [C, C], BF16, kind="ExternalInput").ap()
    wk = nc.dram_tensor("wk", [C, C], BF16, kind="ExternalInput").ap()
    wv = nc.dram_tensor("wv", [C, C], BF16, kind="ExternalInput").ap()
    wp = nc.dram_tensor("wp", [C, C], BF16, kind="ExternalInput").ap()
    w1 = nc.dram_tensor("w1", [C, FFN], BF16, kind="ExternalInput").ap()
    w2 = nc.dram_tensor("w2", [FFN, C], BF16, kind="ExternalInput").ap()
    biasT = nc.dram_tensor("biasT", [128, 3, 64], F32, kind="ExternalInput").ap()
    e6hi = nc.dram_tensor("e6hi", [6, 128], BF16, kind="ExternalInput").ap()
    e6lo = nc.dram_tensor("e6lo", [6, 64], BF16, kind="ExternalInput").ap()
    ones6 = nc.dram_tensor("ones6", [128, 3, 6], BF16, kind="ExternalInput").ap()
    ident = nc.dram_tensor("ident", [128, 128], BF16, kind="ExternalInput").ap()
    out = nc.dram_tensor("out", [N_WIN * 64, C], F32, kind="ExternalOutput").ap()

    with tile.TileContext(nc) as tc:
        kernel_body(tc, xs, ys, wq, wk, wv, wp, w1, w2, biasT, e6hi, e6lo,
                    ones6, ident, out)
    nc.compile()
    return nc


def kernel_body(tc, xs, ys, wq, wk, wv, wp, w1, w2, biasT, e6hi, e6lo, ones6,
                ident, out):
    nc = tc.nc
    ctx_pools = []

    def pool(name, bufs, space="SBUF"):
        p = tc.tile_pool(name=name, bufs=bufs, space=space)
        ctx_pools.append(p)
        return p.__enter__()

    singles = pool("singles", 1)
    sb = pool("sb", 2)
    sb3 = pool("sb3", 3)
    # PSUM budget is 8 banks of [128, 512]xf32; pools allocate statically
    # per tag, so every psum tile shares one of four tags.
    ps_mm = pool("ps_mm", 2, space="PSUM")    # linear-layer outputs (2 banks)
    ps_att = pool("ps_att", 3, space="PSUM")  # scores / attn-out (3 banks)
    ps_r = pool("ps_r", 1, space="PSUM")      # softmax sums + broadcast (1)
    ps_tp = pool("ps_tp", 2, space="PSUM")    # PE-transpose staging (2)

    # ---- static weights/constants ----
    def load_const(name, src_ap, shape, dtype):
        t = singles.tile(shape, dtype, tag=name)
        nc.sync.dma_start(out=t, in_=src_ap)
        return t

    wq_hi = load_const("wq_hi", wq[0:128, :], [128, C], BF16)
    wq_lo = load_const("wq_lo", wq[128:192, :], [64, C], BF16)
    wk_hi = load_const("wk_hi", wk[0:128, :], [128, C], BF16)
    wk_lo = load_const("wk_lo", wk[128:192, :], [64, C], BF16)
    wv_hi = load_const("wv_hi", wv[0:128, :], [128, C], BF16)
    wv_lo = load_const("wv_lo", wv[128:192, :], [64, C], BF16)
    wp_hi = load_const("wp_hi", wp[0:128, :], [128, C], BF16)
    wp_lo = load_const("wp_lo", wp[128:192, :], [64, C], BF16)
    w1_hi = load_const("w1_hi", w1[0:128, :], [128, FFN], BF16)
    w1_lo = load_const("w1_lo", w1[128:192, :], [64, FFN], BF16)
    w2_sb = load_const("w2_sb", w2.rearrange("(k p) c -> p k c", p=128), [128, 6, C], BF16)
    biasT_sb = load_const("biasT_sb", biasT, [128, 3, 64], F32)
    ones6_sb = load_const("ones6_sb", ones6, [128, 3, 6], BF16)
    ident_sb = load_const("ident_sb", ident, [128, 128], BF16)
    eps_sb = singles.tile([128, 1], F32, tag="eps")
    nc.vector.memset(eps_sb, 1e-5)

    def layernorm_to(dst_bf16, src_f32, u_pool):
        """dst = (src - mean)/sqrt(var+eps), rowwise over free dim (C)."""
        st = u_pool.tile([128, 6], F32, tag="ln_st")
        mv = u_pool.tile([128, 2], F32, tag="ln_mv")
        nc.vector.bn_stats(out=st, in_=src_f32)
        nc.vector.bn_aggr(out=mv, in_=st)
        nc.scalar.activation(out=mv[:, 1:2], in_=mv[:, 1:2], func=AF.Sqrt,
                             bias=eps_sb, scale=1.0)
        nc.vector.reciprocal(out=mv[:, 1:2], in_=mv[:, 1:2])
        nc.vector.tensor_scalar(out=dst_bf16, in0=src_f32,
                                scalar1=mv[:, 0:1], scalar2=mv[:, 1:2],
                                op0=ALU.subtract, op1=ALU.mult)

    def transpose_to(dst_hi, dst_lo, src_bf16, u):
        """PE-transpose [128,192] bf16 -> columns 128u..128u+128 of
        feature-major dst_hi [128,512], dst_lo [64,512]."""
        t1 = ps_tp.tile([128, 128], BF16, tag="tp")
        nc.tensor.transpose(t1, src_bf16[:, 0:128], ident_sb)
        nc.scalar.activation(out=dst_hi[:, 128 * u:128 * u + 128], in_=t1, func=AF.Copy)
        t2 = ps_tp.tile([128, 128], BF16, tag="tp")
        nc.tensor.transpose(t2[0:64, :], src_bf16[:, 128:192], ident_sb)
        nc.scalar.activation(out=dst_lo[:, 128 * u:128 * u + 128], in_=t2[0:64, :], func=AF.Copy)

    for g in range(N_GROUPS):
        # ---------- Phase A: load + LN1 + transpose ----------
        x_keep = sb.tile([128, 4, C], F32, tag="x_keep")
        xnT_hi = sb.tile([128, 512], BF16, tag="xnT_hi")
        xnT_lo = sb.tile([64, 512], BF16, tag="xnT_lo")
        ynT_hi = sb.tile([128, 512], BF16, tag="ynT_hi")
        ynT_lo = sb.tile([64, 512], BF16, tag="ynT_lo")
        for u in range(4):
            nc.sync.dma_start(out=x_keep[:, u, :], in_=_win_src_ap(xs, g, u))
            xn_t = sb3.tile([128, C], BF16, tag="xn_t")
            layernorm_to(xn_t, x_keep[:, u, :], sb3)
            transpose_to(xnT_hi, xnT_lo, xn_t, u)
            y_t = sb3.tile([128, C], F32, tag="y_t")
            nc.sync.dma_start(out=y_t, in_=_win_src_ap(ys, g, u))
            yn_t = sb3.tile([128, C], BF16, tag="yn_t")
            layernorm_to(yn_t, y_t, sb3)
            transpose_to(ynT_hi, ynT_lo, yn_t, u)

        # ---------- Phase B: q/k feature-major, v token-major ----------
        def linear_fm(dst_hi, dst_lo, lhs_hi, lhs_lo, rhs_hi, rhs_lo):
            """dst[o, n] = sum_c lhs[c, o] * rhs[c, n], o in 0..191."""
            p_hi = ps_mm.tile([128, 512], F32, tag="mm")
            nc.tensor.matmul(p_hi, lhs_hi[:, 0:128], rhs_hi, start=True, stop=False)
            nc.tensor.matmul(p_hi, lhs_lo[:, 0:128], rhs_lo, start=False, stop=True)
            nc.scalar.activation(out=dst_hi, in_=p_hi, func=AF.Copy)
            p_lo = ps_mm.tile([128, 512], F32, tag="mm")
            p_lo = p_lo[0:64, :]
            nc.tensor.matmul(p_lo, lhs_hi[:, 128:192], rhs_hi, start=True, stop=False)
            nc.tensor.matmul(p_lo, lhs_lo[:, 128:192], rhs_lo, start=False, stop=True)
            nc.scalar.activation(out=dst_lo, in_=p_lo, func=AF.Copy)

        q_hi = sb.tile([128, 512], BF16, tag="q_hi")
        q_lo = sb.tile([64, 512], BF16, tag="q_lo")
        linear_fm(q_hi, q_lo, wq_hi, wq_lo, ynT_hi, ynT_lo)
        k_hi = sb.tile([128, 512], BF16, tag="k_hi")
        k_lo = sb.tile([64, 512], BF16, tag="k_lo")
        linear_fm(k_hi, k_lo, wk_hi, wk_lo, xnT_hi, xnT_lo)

        vd = []
        for u in range(4):
            v_ps = ps_mm.tile([128, 512], F32, tag="mm", name=f"v_ps_{g}_{u}")
            v_ps = v_ps[:, 0:C]
            nc.tensor.matmul(v_ps, xnT_hi[:, 128 * u:128 * u + 128], wv_hi,
                             start=True, stop=False)
            nc.tensor.matmul(v_ps, xnT_lo[:, 128 * u:128 * u + 128], wv_lo,
                             start=False, stop=True)
            v_sb = sb.tile([128, C], BF16, tag=f"v_sb{u}")
            nc.scalar.activation(out=v_sb, in_=v_ps, func=AF.Copy)
            # duplicate each window onto both partition slots for AV
            vA = sb.tile([128, C], BF16, tag=f"vdA{u}")
            vB = sb.tile([128, C], BF16, tag=f"vdB{u}")
            nc.sync.dma_start(out=vA[0:64, :], in_=v_sb[0:64, :])
            nc.sync.dma_start(out=vA[64:128, :], in_=v_sb[0:64, :])
            nc.sync.dma_start(out=vB[0:64, :], in_=v_sb[64:128, :])
            nc.sync.dma_start(out=vB[64:128, :], in_=v_sb[64:128, :])
            vd += [vA, vB]

        # ---------- attention ----------
        # scores^T tiles: t = h//2, partition slot s = h%2, cols 64c..64c+64
        attn = []
        for t in range(3):
            sc_t = ps_att.tile([128, 512], F32, tag="att", name=f"sc{t}_{g}")
            for h in (2 * t, 2 * t + 1):
                s = h % 2
                if h < 4:
                    k_sl, q_sl = k_hi, q_hi
                    off = 32 * h
                else:
                    k_sl, q_sl = k_lo, q_lo
                    off = 32 * (h - 4)
                for c in range(8):
                    nc.tensor.matmul(
                        sc_t[64 * s:64 * s + 64, 64 * c:64 * c + 64],
                        k_sl[off:off + 32, 64 * c:64 * c + 64],
                        q_sl[off:off + 32, 64 * c:64 * c + 64],
                        start=True, stop=True,
                        tile_position=(off, 64 * s),
                    )
            sc_v = sc_t.rearrange("p (c n) -> p c n", n=64)
            nc.vector.tensor_tensor(
                out=sc_v, in0=sc_v,
                in1=biasT_sb[:, t:t + 1, :].broadcast_to([128, 8, 64]),
                op=ALU.add)
            a_t = sb.tile([128, 512], BF16, tag=f"attn{t}", name=f"attn{t}_{g}")
            nc.scalar.activation(out=a_t, in_=sc_t, func=AF.Exp)
            attn.append(a_t)

        # row sums -> [6, 512] via indicator ones-matmuls, then 1/x
        r6_ps = ps_r.tile([6, 512], F32, tag="r")
        for t in range(3):
            nc.tensor.matmul(r6_ps, ones6_sb[:, t, :], attn[t],
                             start=(t == 0), stop=(t == 2))
        r6_sb = sb.tile([6, 512], BF16, tag="r6_sb")
        with nc.allow_low_precision(reason="softmax 1/sum broadcast in bf16"):
            nc.vector.reciprocal(out=r6_sb, in_=r6_ps)


        # AV: out^T[32h+d, 64c+i]
        ao_hi = ps_att.tile([128, 512], F32, tag="att")
        ao_lo = ps_att.tile([128, 512], F32, tag="att")
        ao_lo = ao_lo[0:64, :]
        for h in range(NH):
            t, s = h // 2, h % 2
            dst, off = (ao_hi, 32 * h) if h < 4 else (ao_lo, 32 * (h - 4))
            for c in range(8):
                nc.tensor.matmul(
                    dst[off:off + 32, 64 * c:64 * c + 64],
                    vd[c][64 * s:64 * s + 64, 32 * h:32 * h + 32],
                    attn[t][64 * s:64 * s + 64, 64 * c:64 * c + 64],
                    start=True, stop=True,
                    tile_position=(64 * s, off),
                )
        rbc_hi = sb.tile([128, 512], BF16, tag="rbc_hi")
        nc.sync.dma_start(out=rbc_hi,
                          in_=r6_sb[0:4, :].unsqueeze(1).broadcast_to([4, 32, 512]))
        rbc_lo = sb.tile([64, 512], BF16, tag="rbc_lo")
        nc.sync.dma_start(out=rbc_lo,
                          in_=r6_sb[4:6, :].unsqueeze(1).broadcast_to([2, 32, 512]))
        aoT_hi = sb.tile([128, 512], BF16, tag="aoT_hi")
        nc.vector.tensor_tensor(out=aoT_hi, in0=ao_hi, in1=rbc_hi, op=ALU.mult)
        aoT_lo = sb.tile([64, 512], BF16, tag="aoT_lo")
        nc.vector.tensor_tensor(out=aoT_lo, in0=ao_lo, in1=rbc_lo, op=ALU.mult)

        # ---------- proj + residual + LN2 ----------
        poT_hi = sb.tile([128, 512], BF16, tag="poT_hi")
        poT_lo = sb.tile([64, 512], BF16, tag="poT_lo")
        linear_fm(poT_hi, poT_lo, wp_hi, wp_lo, aoT_hi, aoT_lo)

        x2 = sb.tile([128, 4, C], F32, tag="x2")
        xn2T_hi = sb.tile([128, 512], BF16, tag="xn2T_hi")
        xn2T_lo = sb.tile([64, 512], BF16, tag="xn2T_lo")
        for u in range(4):
            t1 = ps_tp.tile([128, 128], BF16, tag="tp")
            nc.tensor.transpose(t1, poT_hi[:, 128 * u:128 * u + 128], ident_sb)
            t2 = ps_tp.tile([128, 128], BF16, tag="tp")
            t2 = t2[:, 0:64]
            nc.tensor.transpose(t2, poT_lo[:, 128 * u:128 * u + 128],
                                ident_sb[0:64, 0:64])
            nc.vector.tensor_tensor(out=x2[:, u, 0:128], in0=t1,
                                    in1=x_keep[:, u, 0:128], op=ALU.add)
            nc.vector.tensor_tensor(out=x2[:, u, 128:192], in0=t2,
                                    in1=x_keep[:, u, 128:192], op=ALU.add)
            xn2_t = sb3.tile([128, C], BF16, tag="xn2_t")
            layernorm_to(xn2_t, x2[:, u, :], sb3)
            transpose_to(xn2T_hi, xn2T_lo, xn2_t, u)

        # ---------- MLP ----------
        hT = sb.tile([128, 6, 512], BF16, tag="hT")
        for m in range(6):
            h_ps = ps_mm.tile([128, 512], F32, tag="mm", name=f"h_ps_{g}_{m}")
            nc.tensor.matmul(h_ps, w1_hi[:, 128 * m:128 * m + 128], xn2T_hi,
                             start=True, stop=False)
            nc.tensor.matmul(h_ps, w1_lo[:, 128 * m:128 * m + 128], xn2T_lo,
                             start=False, stop=True)
            nc.scalar.activation(out=hT[:, m, :], in_=h_ps, func=AF.Gelu)

        po2_hi = ps_mm.tile([128, 512], F32, tag="mm")
        po2_lo = ps_mm.tile([128, 512], F32, tag="mm")
        po2_lo = po2_lo[0:64, :]
        for kc in range(6):
            nc.tensor.matmul(po2_hi, w2_sb[:, kc, 0:128], hT[:, kc, :],
                             start=(kc == 0), stop=(kc == 5))
        for kc in range(6):
            nc.tensor.matmul(po2_lo, w2_sb[:, kc, 128:192], hT[:, kc, :],
                             start=(kc == 0), stop=(kc == 5))
        po2T_hi = sb.tile([128, 512], BF16, tag="po2T_hi")
        nc.scalar.activation(out=po2T_hi, in_=po2_hi, func=AF.Copy)
        po2T_lo = sb.tile([64, 512], BF16, tag="po2T_lo")
        nc.scalar.activation(out=po2T_lo, in_=po2_lo, func=AF.Copy)

        # ---------- final residual + store ----------
        for u in range(4):
            t1 = ps_tp.tile([128, 128], BF16, tag="tp")
            nc.tensor.transpose(t1, po2T_hi[:, 128 * u:128 * u + 128], ident_sb)
            t2 = ps_tp.tile([128, 128], BF16, tag="tp")
            t2 = t2[:, 0:64]
            nc.tensor.transpose(t2, po2T_lo[:, 128 * u:128 * u + 128],
                                ident_sb[0:64, 0:64])
            o_t = sb3.tile([128, C], F32, tag="o_t")
            nc.vector.tensor_tensor(out=o_t[:, 0:128], in0=t1,
                                    in1=x2[:, u, 0:128], op=ALU.add)
            nc.vector.tensor_tensor(out=o_t[:, 128:192], in0=t2,
                                    in1=x2[:, u, 128:192], op=ALU.add)
            nc.sync.dma_start(out=_win_src_ap(out, g, u), in_=o_t)

    for p in reversed(ctx_pools):
        p.__exit__(None, None, None)


def prep_inputs(inputs):
    """Host-side prep: fold norms/scales into weights, build constants, slab inputs."""
    f32 = lambda a: np.ascontiguousarray(np.asarray(a, np.float32))
    x, y = f32(inputs['x']), f32(inputs['y'])
    qkv_w, qkv_b = f32(inputs['qkv_w']), f32(inputs['qkv_b'])
    g1, b1n = f32(inputs['norm1_g']), f32(inputs['norm1_b'])
    g2, b2n = f32(inputs['norm2_g']), f32(inputs['norm2_b'])

    # fold norm1 gamma/beta into qkv (biases are zero in this problem; assert)
    wq_eff = g1[:, None] * qkv_w[:, 0:C] * SCALE
    wk_eff = g1[:, None] * qkv_w[:, C:2 * C]
    wv_eff = g1[:, None] * qkv_w[:, 2 * C:]
    bq = b1n @ qkv_w[:, 0:C] * SCALE + qkv_b[0:C] * SCALE
    bk = b1n @ qkv_w[:, C:2 * C] + qkv_b[C:2 * C]
    bv = b1n @ qkv_w[:, 2 * C:] + qkv_b[2 * C:]
    w1_eff = g2[:, None] * f32(inputs['fc1_w'])
    b1_eff = b2n @ f32(inputs['fc1_w']) + f32(inputs['fc1_b'])
    assert not (np.any(bq) or np.any(bk) or np.any(bv) or np.any(b1_eff) or
                np.any(f32(inputs['proj_b'])) or np.any(f32(inputs['fc2_b']))), \
        "nonzero biases not folded in this build"

    rel = _rel_index()
    rpb = f32(inputs['rpb_table'])
    bias_full = rpb[rel]                     # [n, m, NH]
    biasT = np.zeros((128, 3, 64), np.float32)
    for h in range(NH):
        t, s = h // 2, h % 2
        biasT[64 * s:64 * s + 64, t, :] = bias_full[:, :, h].T  # [m, n]

    e6hi = np.zeros((6, 128), np.float32)
    for h in range(4):
        e6hi[h, 32 * h:32 * h + 32] = 1.0
    e6lo = np.zeros((6, 64), np.float32)
    for h in (4, 5):
        e6lo[h, 32 * (h - 4):32 * (h - 4) + 32] = 1.0
    ones6 = np.zeros((128, 3, 6), np.float32)
    for t in range(3):
        for s in range(2):
            ones6[64 * s:64 * s + 64, t, 2 * t + s] = 1.0

    shared = {
        'wq': bf16(wq_eff), 'wk': bf16(wk_eff), 'wv': bf16(wv_eff),
        'wp': bf16(inputs['proj_w']), 'w1': bf16(w1_eff),
        'w2': bf16(inputs['fc2_w']),
        'biasT': biasT, 'e6hi': bf16(e6hi), 'e6lo': bf16(e6lo),
        'ones6': bf16(ones6), 'ident': bf16(np.eye(128)),
    }
    in_maps = []
    for i in range(N_CORES):
        m = dict(shared)
        m['xs'] = win_permute(x[0, i * DS:(i + 1) * DS])
        m['ys'] = win_permute(y[0, i * DS:(i + 1) * DS])
        in_maps.append(m)
    return in_maps


_CACHED_NC = None


def get_program(in_maps):
    global _CACHED_NC
    if _CACHED_NC is None:
        _CACHED_NC = build_program(in_maps[0])
    return _CACHED_NC


def kernel(**inputs):
    in_maps = prep_inputs(inputs)
    nc = get_program(in_maps)
    res = run_bass_kernel_spmd(nc, in_maps, list(range(N_CORES)))
    outs = [win_unpermute(res.results[i]["out"]) for i in range(N_CORES)]
    full = np.concatenate([o[None] for o in outs], axis=0)  # [8, DS, H, W, C]
    full = full.reshape(1, D, H, W, C).astype(np.float32)
    return full

